# revision 1
# baseline (speedup 1.0000x reference)
"""ChannelAttentionPropagation1D kernel for 8x TRN2 NeuronCores.

Reference computation (per batch b):
  kv[c,d]   = sum_{t,n} key_mem[b,t,n,c] * val_mem[b,t,n,d]    # (64, 64)
  kv_soft   = softmax(kv, axis=c)
  out[n,d]  = alpha * (key_cur[b] @ kv_soft)[n,d] + val_cur[b,n,d]

Sharding (8 cores):
  phase 1: core i contracts the t=i slice of key_mem/val_mem (16384 tokens
           per batch) into a partial kv^T, then AllReduce (64 KB) over cores.
  phase 2: core i computes the n-slice [2048*i, 2048*(i+1)) of the output.

Layout notes:
  - phase 1 accumulates kvT[d,c] (PSUM) so the softmax axis c lands on the
    free axis; a tiny PE transpose afterwards yields kv_soft[c,d].
  - key_cur is transposed (and scaled by alpha) on the host so its channel
    axis is the SBUF partition axis; its token axis is permuted n = 16p + j
    so phase-2 output tiles assemble into 4KB-contiguous-per-partition
    stores.
"""

import numpy as np

import concourse.bacc as bacc
import concourse.mybir as mybir
import concourse.tile as tile
from concourse import bass_utils, masks

F32 = mybir.dt.float32

N_CORES = 8
N, T, NTOK, C, C2 = 4, 8, 16384, 64, 64
NSL = NTOK // N_CORES  # 2048: phase-2 token slice per core
A_TILES = 64           # 128-token matmul tiles per half-batch chunk
HALF = NTOK // 2       # 8192 tokens per phase-1 DMA chunk

_CACHE = {}

# Extra kwargs forwarded to run_bass_kernel_spmd (used by the profiling
# harness to request an NTFF trace; empty for normal correctness runs).
_RUN_OPTS = {}


def _build_program():
    nc = bacc.Bacc(
        "TRN2",
        target_bir_lowering=False,
        debug=False,
        enable_asserts=False,
        num_devices=N_CORES,
    )

    km = nc.dram_tensor("key_mem", [N, NTOK, C], F32, kind="ExternalInput").ap()
    vm = nc.dram_tensor("val_mem", [N, NTOK, C2], F32, kind="ExternalInput").ap()
    # key_curT is host-packed [128, NSL/2]: rows 0:64 = channels for output
    # tiles j=0..7, rows 64:128 = channels for tiles j=8..15 (row-tiled
    # phase-2 pairs).
    kct = nc.dram_tensor(
        "key_curT", [N, 128, NSL // 2], F32, kind="ExternalInput"
    ).ap()
    vc = nc.dram_tensor("val_cur", [N, NSL, C2], F32, kind="ExternalInput").ap()
    out = nc.dram_tensor("out", [N, NSL, C2], F32, kind="ExternalOutput").ap()

    with tile.TileContext(nc) as tc:
        with (
            tc.tile_pool(name="persist", bufs=1) as persist,
            tc.tile_pool(name="big", bufs=4) as big,
            tc.tile_pool(name="tmp", bufs=2) as tmp,
            tc.tile_pool(name="stage", bufs=2) as stage_pool,
            tc.tile_pool(name="ps", bufs=2, space="PSUM") as ps,
            tc.tile_pool(name="dram", bufs=1, space="DRAM") as dram,
        ):
            ident = persist.tile([128, 128], F32)
            masks.make_identity(nc, ident[:])

            kct_sb = persist.tile([128, N * (NSL // 2)], F32)
            vc_sb = persist.tile([128, N * (NSL // 128) * C2], F32)

            kvt_sb = persist.tile([C2, N * C], F32)
            kvt_all = persist.tile([C2, N * N_CORES * C], F32)
            kvt_red = persist.tile([C2, N * C], F32)
            kv_soft = persist.tile([128, N * C2], F32)
            ar_outs = {}

            def emit_tails():
                """AR readbacks + softmax + transpose + phase 2 + stores for
                all batches, emitted STAGE-MAJOR: engine FIFOs run in program
                order, so batch-major emission would serialize the four
                ~15us-latency chains. Stage-major lets the four batches
                pipeline through gpsimd/DVE/ACT/PE. All tails sit after the
                whole phase-1 so a late AllReduce (peer-core launch skew can
                exceed 100us) never blocks local phase-1 work."""
                # readbacks ride the sync queue: its chunk DMAs have drained
                # by now, while gpsimd still holds doorbell-3 (which waits
                # for the end of phase-1) and scalar holds ar_in3. Each
                # AllGather result is [rank, d, c]; pull it into SBUF as
                # [d, (rank c)] and tree-reduce with 3 DVE adds per batch.
                W = N_CORES * C
                for b in range(N):
                    nc.sync.dma_start(
                        kvt_all[:, b * W:(b + 1) * W].rearrange(
                            "d (r c) -> d r c", r=N_CORES
                        ),
                        ar_outs[b].rearrange("r d c -> d r c"),
                    )
                for width in (4 * C, 2 * C):
                    for b in range(N):
                        lo = kvt_all[:, b * W: b * W + width]
                        nc.vector.tensor_add(
                            lo, lo, kvt_all[:, b * W + width: b * W + 2 * width]
                        )
                for b in range(N):
                    nc.vector.tensor_add(
                        kvt_red[:, b * C:(b + 1) * C],
                        kvt_all[:, b * W: b * W + C],
                        kvt_all[:, b * W + C: b * W + 2 * C],
                    )
                neg_mx = tmp.tile([C2, N], F32)
                for b in range(N):
                    nc.vector.reduce_max(
                        out=neg_mx[:, b:b + 1],
                        in_=kvt_red[:, b * C:(b + 1) * C],
                        axis=mybir.AxisListType.X,
                        negate=True,
                    )
                ex = tmp.tile([C2, N * C], F32)
                sm = tmp.tile([C2, N], F32)
                for b in range(N):
                    nc.scalar.activation(
                        ex[:, b * C:(b + 1) * C],
                        kvt_red[:, b * C:(b + 1) * C],
                        mybir.ActivationFunctionType.Exp,
                        bias=neg_mx[:, b:b + 1], scale=1.0,
                        accum_out=sm[:, b:b + 1],
                    )
                rv = tmp.tile([C2, N], F32)
                for b in range(N):
                    nc.vector.reciprocal(rv[:, b:b + 1], sm[:, b:b + 1])
                for b in range(N):
                    nc.vector.tensor_scalar_mul(
                        ex[:, b * C:(b + 1) * C],
                        ex[:, b * C:(b + 1) * C],
                        rv[:, b:b + 1],
                    )
                # Transpose softmaxed kvT to kv[c, d] (transpose-mode matmul
                # must write PSUM partition 0), then mirror the whole strip
                # into partitions 64:128 with one SBUF->SBUF DMA so row-tiled
                # phase-2 can read kv from the upper rows too.
                for b in range(N):
                    tp = ps.tile([C, C2], F32, tag="tp", name=f"tp{b}", bufs=2)
                    nc.tensor.transpose(
                        tp[:], ex[:, b * C:(b + 1) * C], ident[0:C2, 0:C2]
                    )
                    nc.vector.tensor_copy(
                        kv_soft[0:C, b * C2:(b + 1) * C2], tp[:]
                    )
                nc.sync.dma_start(kv_soft[64:64 + C, :], kv_soft[0:C, :])
                stgs = {}
                for b in range(N):
                    stgs[b] = stage_pool.tile(
                        [128, (NSL // 128) * C2], F32, tag=f"stg{b}",
                        name=f"stg{b}",
                    )
                # Row-tiled phase 2: tile j contracts on PE rows 0:64
                # (kct rows 0:64, kv rows 0:64), tile j+8 on rows 64:128 —
                # the two matmuls run concurrently on separate subarrays.
                HNSL = NSL // 2
                for b in range(N):
                    for j in range(8):
                        col = slice(b * HNSL + j * 128, b * HNSL + (j + 1) * 128)
                        o_a = ps.tile(
                            [128, C2], F32, tag="o", name=f"oa{b}_{j}", bufs=4
                        )
                        nc.tensor.matmul(
                            o_a[:],
                            lhsT=kct_sb[0:C, col],
                            rhs=kv_soft[0:C, b * C2:(b + 1) * C2],
                            start=True,
                            stop=True,
                            tile_position=(0, 0),
                        )
                        o_b = ps.tile(
                            [128, C2], F32, tag="o", name=f"ob{b}_{j}", bufs=4
                        )
                        nc.tensor.matmul(
                            o_b[:],
                            lhsT=kct_sb[64:64 + C, col],
                            rhs=kv_soft[64:64 + C, b * C2:(b + 1) * C2],
                            start=True,
                            stop=True,
                            tile_position=(64, 0),
                        )
                        nc.vector.tensor_add(
                            stgs[b][:, j * C2:(j + 1) * C2],
                            o_a[:],
                            vc_sb[:, b * 1024 + j * C2: b * 1024 + (j + 1) * C2],
                        )
                        nc.vector.tensor_add(
                            stgs[b][:, (j + 8) * C2:(j + 9) * C2],
                            o_b[:],
                            vc_sb[:, b * 1024 + (j + 8) * C2: b * 1024 + (j + 9) * C2],
                        )
                    # split the store so the second half overlaps the
                    # remaining adds (trims the last batch's tail)
                    oap = out[b].rearrange("(p j) c -> p (j c)", p=128)
                    nc.sync.dma_start(oap[:, 0:8 * C2], stgs[b][:, 0:8 * C2])
                    nc.sync.dma_start(
                        oap[:, 8 * C2:16 * C2], stgs[b][:, 8 * C2:16 * C2]
                    )

            # ---- phase 1: partial kvT[d, c] per batch, col-tiled 2x ----
            # Even token-tiles accumulate on PE column group 0 (psum rows
            # 0:64), odd tiles on column group 2 (psum rows 64:128); the two
            # halves' LDWEIGHTS/MATMUL overlap on independent subarrays.
            for b in range(N):
                kv_ps = ps.tile([128, C], F32, tag="kv", name=f"kv{b}")
                for h in range(2):
                    k_sb = big.tile([128, HALF // 128 * C], F32, tag="k")
                    v_sb = big.tile([128, HALF // 128 * C2], F32, tag="v")
                    sl = slice(h * HALF, (h + 1) * HALF)
                    nc.sync.dma_start(
                        k_sb[:], km[b, sl, :].rearrange("(p a) c -> p (a c)", p=128)
                    )
                    nc.sync.dma_start(
                        v_sb[:], vm[b, sl, :].rearrange("(p a) c -> p (a c)", p=128)
                    )
                    if h == 1:
                        # phase-2 inputs for batch b: issued on the scalar
                        # (ACT) DMA FIFO so they never delay the phase-1
                        # chunk stream on the sync FIFO.
                        nc.scalar.dma_start(
                            kct_sb[:, b * (NSL // 2):(b + 1) * (NSL // 2)],
                            kct[b],
                        )
                        nc.scalar.dma_start(
                            vc_sb[:, b * 1024:(b + 1) * 1024],
                            vc[b].rearrange("(p j) c -> p (j c)", p=128),
                        )
                    for a in range(A_TILES):
                        half = a % 2
                        nc.tensor.matmul(
                            kv_ps[64 * half:64 * half + C2, :],
                            lhsT=v_sb[:, a * C2:(a + 1) * C2],
                            rhs=k_sb[:, a * C:(a + 1) * C],
                            start=(h == 0 and a < 2),
                            stop=(h == 1 and a >= A_TILES - 2),
                            tile_position=(0, 64 * half),
                        )
                # partial kvT = even-half + odd-half (DVE can read only one
                # PSUM operand per instruction, so copy then add)
                nc.vector.tensor_copy(kvt_sb[:, b * C:(b + 1) * C], kv_ps[0:C2, :])
                nc.vector.tensor_add(
                    kvt_sb[:, b * C:(b + 1) * C],
                    kvt_sb[:, b * C:(b + 1) * C],
                    kv_ps[64:64 + C2, :],
                )
                # per-batch AllGather (cheaper than AllReduce on the CC
                # core); the 8 partials are tree-reduced locally on DVE.
                ar_in = dram.tile([C2, C], F32, tag=f"ar_in{b}", name=f"ar_in{b}")
                ar_out = dram.tile(
                    [N_CORES, C2, C], F32, addr_space="Shared", tag=f"ar_out{b}",
                    name=f"ar_out{b}",
                )
                ar_outs[b] = ar_out
                nc.scalar.dma_start(ar_in[:], kvt_sb[:, b * C:(b + 1) * C])
                nc.gpsimd.collective_compute(
                    "AllGather",
                    mybir.AluOpType.bypass,
                    replica_groups=[list(range(N_CORES))],
                    ins=[ar_in.opt()],
                    outs=[ar_out.opt()],
                )
            emit_tails()

    nc.compile()
    return nc


def _get_program():
    if "nc" not in _CACHE:
        _CACHE["nc"] = _build_program()
    return _CACHE["nc"]


def kernel(key_mem, val_mem, key_cur, val_cur, alpha):
    key_mem = np.asarray(key_mem, dtype=np.float32)
    val_mem = np.asarray(val_mem, dtype=np.float32)
    key_cur = np.asarray(key_cur, dtype=np.float32)
    val_cur = np.asarray(val_cur, dtype=np.float32)
    alpha_f = float(np.asarray(alpha).reshape(-1)[0])

    nc = _get_program()

    # key_cur^T with alpha folded in; token axis permuted so that SBUF
    # column j*128+p holds token p*16+j (phase-2 store contiguity).
    kc_scaled = (alpha_f * key_cur).astype(np.float32)
    in_maps = []
    for i in range(N_CORES):
        kct_i = kc_scaled[:, i * NSL:(i + 1) * NSL, :].transpose(0, 2, 1)
        kct_i = (
            kct_i.reshape(N, C, 128, NSL // 128)
            .transpose(0, 1, 3, 2)
            .reshape(N, C, NSL)
        )
        # pack for row-tiled phase 2: rows 0:64 = tiles j=0..7,
        # rows 64:128 = tiles j=8..15
        kct_i = (
            kct_i.reshape(N, C, 2, NSL // 2)
            .transpose(0, 2, 1, 3)
            .reshape(N, 128, NSL // 2)
        )
        in_maps.append(
            {
                "key_mem": np.ascontiguousarray(key_mem[:, i]),
                "val_mem": np.ascontiguousarray(val_mem[:, i]),
                "key_curT": np.ascontiguousarray(kct_i),
                "val_cur": np.ascontiguousarray(val_cur[:, i * NSL:(i + 1) * NSL, :]),
            }
        )

    res = bass_utils.run_bass_kernel_spmd(
        nc, in_maps, core_ids=list(range(N_CORES)), **_RUN_OPTS
    )
    _CACHE["last_result"] = res
    outs = [res.results[i]["out"] for i in range(N_CORES)]
    return np.concatenate(outs, axis=1).astype(np.float32)



# revision 4
# speedup vs baseline: 1.0398x; 1.0398x over previous
"""ChannelAttentionPropagation1D kernel for 8x TRN2 NeuronCores.

Reference computation (per batch b):
  kv[c,d]   = sum_{t,n} key_mem[b,t,n,c] * val_mem[b,t,n,d]    # (64, 64)
  kv_soft   = softmax(kv, axis=c)
  out[n,d]  = alpha * (key_cur[b] @ kv_soft)[n,d] + val_cur[b,n,d]

Sharding (8 cores, pair-per-batch):
  core i owns batch b = i//2, token half h = i%2.
  phase 1: core i contracts its 65536 memory tokens into a partial
           kvT[d,c]; ONE pair AllGather (16 KB) merges the two halves.
  phase 2: core i computes the n-slice [h*8192, (h+1)*8192) of batch b.

Layout notes:
  - key/val memory tokens are host-interleaved into one packed stream
    [128, 512*128] (per 128-token tile: 64 key cols then 64 val cols) so
    one DMA feeds both matmul operands; 2 MiB chunks alternate between
    the two HWDGE queues (sync / scalar) to overlap DMA fixed costs.
  - phase 1 accumulates kvT[d,c] in PSUM col-tiled 2x (even tiles on PE
    column group 0, odd on group 2) so LDWEIGHTS/MATMUL overlap.
  - phase 2 computes out^T[d, tok] with kv_soft stationary (loaded once
    per column group) and alpha-folded key_cur^T as the N=512 moving
    operand; token halves A/B land on PSUM partitions 0:64 / 64:128 of
    one bank via column groups 0/2, so a single [128, 512] DVE add folds
    val_cur in. NOTE: matmuls must write PSUM at column offset 0 —
    column-offset PSUM writes crash the hardware.
"""

import numpy as np

import concourse.bacc as bacc
import concourse.mybir as mybir
import concourse.tile as tile
from concourse import bass_utils, masks

F32 = mybir.dt.float32

N_CORES = 8
N, T, NTOK, C, C2 = 4, 8, 16384, 64, 64
NT1 = 512          # phase-1 128-token matmul tiles per core
NSL = 8192         # phase-2 token slice per core
HSL = NSL // 2     # 4096 tokens per phase-2 half
CHUNK_TILES = 32   # phase-1 tiles per DMA chunk (32 * 128 cols * 4B = 2 MiB)
N_CHUNKS = NT1 // CHUNK_TILES

_CACHE = {}

# Extra kwargs forwarded to run_bass_kernel_spmd (used by the profiling
# harness to request an NTFF trace; empty for normal correctness runs).
_RUN_OPTS = {}


def _build_program():
    nc = bacc.Bacc(
        "TRN2",
        target_bir_lowering=False,
        debug=False,
        enable_asserts=False,
        num_devices=N_CORES,
    )

    kvp = nc.dram_tensor("kv_pack", [128, NT1 * 128], F32, kind="ExternalInput").ap()
    kct = nc.dram_tensor("key_curT", [2, C, HSL], F32, kind="ExternalInput").ap()
    vc = nc.dram_tensor("val_cur", [128, HSL], F32, kind="ExternalInput").ap()
    out = nc.dram_tensor("out", [128, HSL], F32, kind="ExternalOutput").ap()

    with tile.TileContext(nc) as tc:
        with (
            tc.tile_pool(name="persist", bufs=1) as persist,
            tc.tile_pool(name="big", bufs=5) as big,
            tc.tile_pool(name="tmp", bufs=2) as tmp,
            tc.tile_pool(name="ps", bufs=2, space="PSUM") as ps,
            tc.tile_pool(name="dram", bufs=1, space="DRAM") as dram,
        ):
            ident = persist.tile([128, 128], F32)
            masks.make_identity(nc, ident[:])

            kct_a = persist.tile([C, HSL], F32)
            kct_b = persist.tile([C, HSL], F32)
            vc_sb = persist.tile([128, HSL], F32)
            stage = persist.tile([128, HSL], F32)

            kvt_sb = persist.tile([C2, C], F32)
            kvt_all = persist.tile([C2, 2 * C], F32)
            kv_soft = persist.tile([C, C2], F32)

            # ---- phase 1: partial kvT[d, c], col-tiled 2x ----
            kv_ps = ps.tile([128, C], F32, tag="kv", bufs=1)
            for ci in range(N_CHUNKS):
                buf = big.tile([128, CHUNK_TILES * 128], F32, tag="k")
                q = nc.sync if ci % 2 == 0 else nc.scalar
                lo = ci * CHUNK_TILES * 128
                q.dma_start(buf[:], kvp[:, lo:lo + CHUNK_TILES * 128])
                if ci == 2:
                    # phase-2 inputs ride the gpsimd (SWDGE) queue so they
                    # never delay the phase-1 chunk stream on the HWDGE
                    # queues.
                    nc.gpsimd.dma_start(kct_a[:], kct[0])
                    nc.gpsimd.dma_start(kct_b[:], kct[1])
                    nc.gpsimd.dma_start(vc_sb[:], vc)
                for la in range(CHUNK_TILES):
                    a = ci * CHUNK_TILES + la
                    half = a % 2
                    col = la * 128
                    nc.tensor.matmul(
                        kv_ps[64 * half:64 * half + C2, :],
                        lhsT=buf[:, col + 64:col + 128],
                        rhs=buf[:, col:col + 64],
                        start=(a < 2),
                        stop=(a >= NT1 - 2),
                        tile_position=(0, 64 * half),
                    )
            # partial kvT = even-half + odd-half (DVE reads only one PSUM
            # operand per instruction, so copy then add)
            nc.vector.tensor_copy(kvt_sb[:], kv_ps[0:C2, :])
            nc.vector.tensor_add(kvt_sb[:], kvt_sb[:], kv_ps[64:64 + C2, :])

            # ---- pair exchange: one 16 KB AllGather within each pair ----
            ar_in = dram.tile([C2, C], F32, tag="ar_in", name="ar_in")
            # pair groups (<=4 cores) require a Local (non-shared) output
            ar_out = dram.tile([2, C2, C], F32, tag="ar_out", name="ar_out")
            nc.scalar.dma_start(ar_in[:], kvt_sb[:])
            nc.gpsimd.collective_compute(
                "AllGather",
                mybir.AluOpType.bypass,
                replica_groups=[[0, 1], [2, 3], [4, 5], [6, 7]],
                ins=[ar_in.opt()],
                outs=[ar_out.opt()],
            )
            nc.sync.dma_start(
                kvt_all[:].rearrange("d (r c) -> d r c", r=2),
                ar_out.rearrange("r d c -> d r c"),
            )
            kvt_red = tmp.tile([C2, C], F32)
            nc.vector.tensor_add(
                kvt_red[:], kvt_all[:, 0:C], kvt_all[:, C:2 * C]
            )

            # ---- softmax over c (free axis of kvT) ----
            neg_mx = tmp.tile([C2, 1], F32)
            nc.vector.reduce_max(
                out=neg_mx[:],
                in_=kvt_red[:],
                axis=mybir.AxisListType.X,
                negate=True,
            )
            ex = tmp.tile([C2, C], F32)
            sm = tmp.tile([C2, 1], F32)
            nc.scalar.activation(
                ex[:], kvt_red[:],
                mybir.ActivationFunctionType.Exp,
                bias=neg_mx[:], scale=1.0,
                accum_out=sm[:],
            )
            rv = tmp.tile([C2, 1], F32)
            nc.vector.reciprocal(rv[:], sm[:])
            nc.vector.tensor_scalar_mul(ex[:], ex[:], rv[:])
            # Transpose softmaxed kvT to kv[c, d] (transpose-mode matmul
            # must write PSUM partition 0).
            tp = ps.tile([C, C2], F32, tag="tp", bufs=1)
            nc.tensor.transpose(tp[:], ex[:], ident[0:C2, 0:C2])
            nc.vector.tensor_copy(kv_soft[:], tp[:])

            # ---- phase 2: out^T[d, tok] = kv_soft^T @ key_cur^T + vc^T ----
            for s in range(8):
                pg = ps.tile([128, 512], F32, tag="o", name=f"o{s}", bufs=4)
                sl = slice(s * 512, (s + 1) * 512)
                nc.tensor.matmul(
                    pg[0:64, :],
                    lhsT=kv_soft[:],
                    rhs=kct_a[:, sl],
                    start=True, stop=True,
                    tile_position=(0, 0),
                )
                nc.tensor.matmul(
                    pg[64:128, :],
                    lhsT=kv_soft[:],
                    rhs=kct_b[:, sl],
                    start=True, stop=True,
                    tile_position=(0, 64),
                )
                nc.vector.tensor_add(stage[:, sl], pg[:], vc_sb[:, sl])
                # store each quarter as it completes; alternate queues so
                # stores overlap the remaining adds
                if s % 2 == 1:
                    q = nc.sync if s % 4 == 1 else nc.scalar
                    lo = (s - 1) * 512
                    q.dma_start(out[:, lo:lo + 1024], stage[:, lo:lo + 1024])

    nc.compile()
    return nc


def _get_program():
    if "nc" not in _CACHE:
        _CACHE["nc"] = _build_program()
    return _CACHE["nc"]


def kernel(key_mem, val_mem, key_cur, val_cur, alpha):
    key_mem = np.asarray(key_mem, dtype=np.float32)
    val_mem = np.asarray(val_mem, dtype=np.float32)
    key_cur = np.asarray(key_cur, dtype=np.float32)
    val_cur = np.asarray(val_cur, dtype=np.float32)
    alpha_f = float(np.asarray(alpha).reshape(-1)[0])

    nc = _get_program()

    kc_scaled = (alpha_f * key_cur).astype(np.float32)
    in_maps = []
    for i in range(N_CORES):
        b, h = i // 2, i % 2
        # phase-1 stream: interleave 128-token key/val tiles
        km = key_mem[b, 4 * h:4 * h + 4].reshape(NT1, 128, C)
        vm = val_mem[b, 4 * h:4 * h + 4].reshape(NT1, 128, C2)
        kv_pack = (
            np.concatenate([km, vm], axis=2)
            .transpose(1, 0, 2)
            .reshape(128, NT1 * 128)
        )
        # phase-2: key_cur^T (alpha folded) split into token halves A/B
        kc = kc_scaled[b, h * NSL:(h + 1) * NSL, :].T  # (C, NSL)
        kct_pack = np.stack([kc[:, 0:HSL], kc[:, HSL:NSL]])
        vcT = val_cur[b, h * NSL:(h + 1) * NSL, :].T  # (C2, NSL)
        vc_pack = np.concatenate([vcT[:, 0:HSL], vcT[:, HSL:NSL]], axis=0)
        in_maps.append(
            {
                "kv_pack": np.ascontiguousarray(kv_pack),
                "key_curT": np.ascontiguousarray(kct_pack),
                "val_cur": np.ascontiguousarray(vc_pack),
            }
        )

    res = bass_utils.run_bass_kernel_spmd(
        nc, in_maps, core_ids=list(range(N_CORES)), **_RUN_OPTS
    )
    _CACHE["last_result"] = res
    full = np.empty((N, NTOK, C2), dtype=np.float32)
    for i in range(N_CORES):
        b, h = i // 2, i % 2
        o = res.results[i]["out"]  # [128, HSL] = out^T halves stacked
        full[b, h * NSL:h * NSL + HSL, :] = o[0:C2].T
        full[b, h * NSL + HSL:(h + 1) * NSL, :] = o[C2:2 * C2].T
    return full


# revision 5
# speedup vs baseline: 1.0870x; 1.0454x over previous
"""ChannelAttentionPropagation1D kernel for 8x TRN2 NeuronCores.

Reference computation (per batch b):
  kv[c,d]   = sum_{t,n} key_mem[b,t,n,c] * val_mem[b,t,n,d]    # (64, 64)
  kv_soft   = softmax(kv, axis=c)
  out[n,d]  = alpha * (key_cur[b] @ kv_soft)[n,d] + val_cur[b,n,d]

Sharding (8 cores, pair-per-batch):
  core i owns batch b = i//2, token half h = i%2.
  phase 1: core i contracts its 65536 memory tokens into a partial
           kvT[d,c]; ONE pair AllGather (16 KB) merges the two halves.
  phase 2: core i computes the n-slice [h*8192, (h+1)*8192) of batch b.

Layout notes:
  - key/val memory tokens are host-interleaved into one packed stream
    [128, 512*128] (per 128-token tile: 64 key cols then 64 val cols) so
    one DMA feeds both matmul operands; 4 MiB chunks alternate between
    the two HWDGE queues (sync / scalar) to overlap DMA fixed costs.
  - phase 1 accumulates kvT[d,c] in PSUM col-tiled 2x (even tiles on PE
    column group 0, odd on group 2) so LDWEIGHTS/MATMUL overlap.
  - phase-2 inputs load AFTER the last phase-1 chunk (they stream during
    the collective wait), with key_cur^T cast to bf16 during the SWDGE
    DMA: bf16 keeps phase-2 matmuls fast even though the PE is
    HAM-cold (1.2 GHz) after idling through the collective.
  - a dummy 256 B pair AllGather fires at kernel start to absorb the
    collective control-plane warmup (ncfw wakeup + SPAD staging).
  - phase 2 computes out^T[d, tok] with kv_soft stationary (loaded once
    per column group) and key_cur^T as the N=512 moving operand; token
    halves A/B land on PSUM partitions 0:64 / 64:128 of one bank via
    column groups 0/2, so a single [128, 512] DVE add folds val_cur in.
    NOTE: matmuls must write PSUM at column offset 0 — column-offset
    PSUM writes crash the hardware.
"""

import numpy as np

import concourse.bacc as bacc
import concourse.mybir as mybir
import concourse.tile as tile
from concourse import bass_utils, masks

F32 = mybir.dt.float32
BF16 = mybir.dt.bfloat16

N_CORES = 8
N, T, NTOK, C, C2 = 4, 8, 16384, 64, 64
NT1 = 512          # phase-1 128-token matmul tiles per core
NSL = 8192         # phase-2 token slice per core
HSL = NSL // 2     # 4096 tokens per phase-2 half
CHUNK_TILES = 64   # phase-1 tiles per DMA chunk (64 * 128 cols * 4B = 4 MiB)
N_CHUNKS = NT1 // CHUNK_TILES
PAIRS = [[0, 1], [2, 3], [4, 5], [6, 7]]

_CACHE = {}

# Extra kwargs forwarded to run_bass_kernel_spmd (used by the profiling
# harness to request an NTFF trace; empty for normal correctness runs).
_RUN_OPTS = {}


def _build_program():
    nc = bacc.Bacc(
        "TRN2",
        target_bir_lowering=False,
        debug=False,
        enable_asserts=False,
        num_devices=N_CORES,
    )

    kvp = nc.dram_tensor("kv_pack", [128, NT1 * 128], F32, kind="ExternalInput").ap()
    kct = nc.dram_tensor("key_curT", [2, C, HSL], F32, kind="ExternalInput").ap()
    vc = nc.dram_tensor("val_cur", [128, HSL], F32, kind="ExternalInput").ap()
    out = nc.dram_tensor("out", [128, HSL], F32, kind="ExternalOutput").ap()

    with tile.TileContext(nc) as tc:
        with (
            tc.tile_pool(name="persist", bufs=1) as persist,
            tc.tile_pool(name="bigA", bufs=2) as bigA,
            tc.tile_pool(name="bigB", bufs=2) as bigB,
            tc.tile_pool(name="tmp", bufs=2) as tmp,
            tc.tile_pool(name="ps", bufs=2, space="PSUM") as ps,
            tc.tile_pool(name="dram", bufs=1, space="DRAM") as dram,
        ):
            ident = persist.tile([128, 128], F32)
            masks.make_identity(nc, ident[:])

            kct_a = persist.tile([C, HSL], BF16)
            kct_b = persist.tile([C, HSL], BF16)
            vc_sb = persist.tile([128, HSL], F32)
            stage = persist.tile([128, HSL], F32)

            kvt_sb = persist.tile([C2, C], F32)
            kvt_all = persist.tile([C2, 2 * C], F32)
            kv_soft = persist.tile([C, C2], BF16)

            # ---- dummy collective: warm the ncfw/SPAD path early so the
            # real exchange doesn't pay first-use latency ----
            warm_in = dram.tile([C2, 1], F32, tag="warm_in", name="warm_in")
            warm_out = dram.tile([2, C2, 1], F32, tag="warm_out", name="warm_out")
            nc.gpsimd.dma_start(warm_in[:], ident[0:C2, 0:1])
            nc.gpsimd.collective_compute(
                "AllGather",
                mybir.AluOpType.bypass,
                replica_groups=PAIRS,
                ins=[warm_in.opt()],
                outs=[warm_out.opt()],
            )

            # ---- phase 1: partial kvT[d, c], col-tiled 2x ----
            kv_ps = ps.tile([128, C], F32, tag="kv", bufs=1)
            for ci in range(N_CHUNKS):
                pool = bigA if ci % 2 == 0 else bigB
                q = nc.sync if ci % 2 == 0 else nc.scalar
                buf = pool.tile([128, CHUNK_TILES * 128], F32, tag="k")
                lo = ci * CHUNK_TILES * 128
                q.dma_start(buf[:], kvp[:, lo:lo + CHUNK_TILES * 128])
                for la in range(CHUNK_TILES):
                    a = ci * CHUNK_TILES + la
                    half = a % 2
                    col = la * 128
                    nc.tensor.matmul(
                        kv_ps[64 * half:64 * half + C2, :],
                        lhsT=buf[:, col + 64:col + 128],
                        rhs=buf[:, col:col + 64],
                        start=(a < 2),
                        stop=(a >= NT1 - 2),
                        tile_position=(0, 64 * half),
                    )
            # phase-2 inputs: issued after the whole phase-1 chunk stream
            # so they never delay it; they ride the gpsimd (SWDGE) queue
            # and stream in during the collective wait. key_cur^T is cast
            # f32 -> bf16 during the DMA.
            nc.gpsimd.dma_start(kct_a[:], kct[0])
            nc.gpsimd.dma_start(kct_b[:], kct[1])
            nc.gpsimd.dma_start(vc_sb[:], vc)

            # partial kvT = even-half + odd-half (DVE reads only one PSUM
            # operand per instruction, so copy then add)
            nc.vector.tensor_copy(kvt_sb[:], kv_ps[0:C2, :])
            nc.vector.tensor_add(kvt_sb[:], kvt_sb[:], kv_ps[64:64 + C2, :])

            # ---- pair exchange: one 16 KB AllGather within each pair ----
            ar_in = dram.tile([C2, C], F32, tag="ar_in", name="ar_in")
            # pair groups (<=4 cores) require a Local (non-shared) output
            ar_out = dram.tile([2, C2, C], F32, tag="ar_out", name="ar_out")
            nc.sync.dma_start(ar_in[:], kvt_sb[:])
            nc.gpsimd.collective_compute(
                "AllGather",
                mybir.AluOpType.bypass,
                replica_groups=PAIRS,
                ins=[ar_in.opt()],
                outs=[ar_out.opt()],
            )
            nc.sync.dma_start(
                kvt_all[:].rearrange("d (r c) -> d r c", r=2),
                ar_out.rearrange("r d c -> d r c"),
            )
            kvt_red = tmp.tile([C2, C], F32)
            nc.vector.tensor_add(
                kvt_red[:], kvt_all[:, 0:C], kvt_all[:, C:2 * C]
            )

            # ---- softmax over c (free axis of kvT) ----
            neg_mx = tmp.tile([C2, 1], F32)
            nc.vector.reduce_max(
                out=neg_mx[:],
                in_=kvt_red[:],
                axis=mybir.AxisListType.X,
                negate=True,
            )
            ex = tmp.tile([C2, C], F32)
            sm = tmp.tile([C2, 1], F32)
            nc.scalar.activation(
                ex[:], kvt_red[:],
                mybir.ActivationFunctionType.Exp,
                bias=neg_mx[:], scale=1.0,
                accum_out=sm[:],
            )
            rv = tmp.tile([C2, 1], F32)
            nc.vector.reciprocal(rv[:], sm[:])
            nc.vector.tensor_scalar_mul(ex[:], ex[:], rv[:])
            # Transpose softmaxed kvT to kv[c, d] (transpose-mode matmul
            # must write PSUM partition 0); the DVE copy casts to bf16.
            tp = ps.tile([C, C2], F32, tag="tp", bufs=1)
            nc.tensor.transpose(tp[:], ex[:], ident[0:C2, 0:C2])
            nc.vector.tensor_copy(kv_soft[:], tp[:])

            # ---- phase 2: out^T[d, tok] = kv_soft^T @ key_cur^T + vc^T ----
            for s in range(8):
                pg = ps.tile([128, 512], F32, tag="o", name=f"o{s}", bufs=4)
                sl = slice(s * 512, (s + 1) * 512)
                nc.tensor.matmul(
                    pg[0:64, :],
                    lhsT=kv_soft[:],
                    rhs=kct_a[:, sl],
                    start=True, stop=True,
                    tile_position=(0, 0),
                )
                nc.tensor.matmul(
                    pg[64:128, :],
                    lhsT=kv_soft[:],
                    rhs=kct_b[:, sl],
                    start=True, stop=True,
                    tile_position=(0, 64),
                )
                nc.vector.tensor_add(stage[:, sl], pg[:], vc_sb[:, sl])
                # store each quarter as it completes; alternate queues so
                # stores overlap the remaining adds
                if s % 2 == 1:
                    q = nc.sync if s % 4 == 1 else nc.scalar
                    lo = (s - 1) * 512
                    q.dma_start(out[:, lo:lo + 1024], stage[:, lo:lo + 1024])

    nc.compile()
    return nc


def _get_program():
    if "nc" not in _CACHE:
        _CACHE["nc"] = _build_program()
    return _CACHE["nc"]


def kernel(key_mem, val_mem, key_cur, val_cur, alpha):
    key_mem = np.asarray(key_mem, dtype=np.float32)
    val_mem = np.asarray(val_mem, dtype=np.float32)
    key_cur = np.asarray(key_cur, dtype=np.float32)
    val_cur = np.asarray(val_cur, dtype=np.float32)
    alpha_f = float(np.asarray(alpha).reshape(-1)[0])

    nc = _get_program()

    kc_scaled = (alpha_f * key_cur).astype(np.float32)
    in_maps = []
    for i in range(N_CORES):
        b, h = i // 2, i % 2
        # phase-1 stream: interleave 128-token key/val tiles
        km = key_mem[b, 4 * h:4 * h + 4].reshape(NT1, 128, C)
        vm = val_mem[b, 4 * h:4 * h + 4].reshape(NT1, 128, C2)
        kv_pack = (
            np.concatenate([km, vm], axis=2)
            .transpose(1, 0, 2)
            .reshape(128, NT1 * 128)
        )
        # phase-2: key_cur^T (alpha folded) split into token halves A/B
        kc = kc_scaled[b, h * NSL:(h + 1) * NSL, :].T  # (C, NSL)
        kct_pack = np.stack([kc[:, 0:HSL], kc[:, HSL:NSL]])
        vcT = val_cur[b, h * NSL:(h + 1) * NSL, :].T  # (C2, NSL)
        vc_pack = np.concatenate([vcT[:, 0:HSL], vcT[:, HSL:NSL]], axis=0)
        in_maps.append(
            {
                "kv_pack": np.ascontiguousarray(kv_pack),
                "key_curT": np.ascontiguousarray(kct_pack),
                "val_cur": np.ascontiguousarray(vc_pack),
            }
        )

    res = bass_utils.run_bass_kernel_spmd(
        nc, in_maps, core_ids=list(range(N_CORES)), **_RUN_OPTS
    )
    _CACHE["last_result"] = res
    full = np.empty((N, NTOK, C2), dtype=np.float32)
    for i in range(N_CORES):
        b, h = i // 2, i % 2
        o = res.results[i]["out"]  # [128, HSL] = out^T halves stacked
        full[b, h * NSL:h * NSL + HSL, :] = o[0:C2].T
        full[b, h * NSL + HSL:(h + 1) * NSL, :] = o[C2:2 * C2].T
    return full


# revision 9
# speedup vs baseline: 1.1282x; 1.0379x over previous
"""ChannelAttentionPropagation1D kernel for 8x TRN2 NeuronCores.

Reference computation (per batch b):
  kv[c,d]   = sum_{t,n} key_mem[b,t,n,c] * val_mem[b,t,n,d]    # (64, 64)
  kv_soft   = softmax(kv, axis=c)
  out[n,d]  = alpha * (key_cur[b] @ kv_soft)[n,d] + val_cur[b,n,d]

Sharding (8 cores, pair-per-batch):
  core i owns batch b = i//2, token half h = i%2.
  phase 1: core i contracts its 65536 memory tokens into a partial
           kvT[d,c]; ONE pair AllGather (16 KB) merges the two halves.
  phase 2: core i computes the n-slice [h*8192, (h+1)*8192) of batch b.

Layout notes:
  - key/val memory tokens are host-interleaved into one packed stream
    [128, 512*128] (per 128-token tile: 64 key cols then 64 val cols) so
    one DMA feeds both matmul operands; 2 MiB chunks alternate between
    the two HWDGE queues (sync / scalar) to overlap DMA fixed costs.
  - phase 1 accumulates kvT[d,c] in PSUM col-tiled 2x (even tiles on PE
    column group 0, odd on group 2) so LDWEIGHTS/MATMUL overlap.
  - phase-2 inputs load AFTER the last phase-1 chunk (they stream during
    the collective wait), with key_cur^T cast to bf16 during the SWDGE
    DMA: bf16 keeps phase-2 matmuls fast even though the PE is
    HAM-cold (1.2 GHz) after idling through the collective.
  - a dummy 256 B pair AllGather fires at kernel start to absorb the
    collective control-plane warmup (ncfw wakeup + SPAD staging).
  - phase 2 computes out^T[d, tok] with kv_soft stationary (loaded once
    per column group) and key_cur^T as the N=512 moving operand; token
    halves A/B land on PSUM partitions 0:64 / 64:128 of one bank via
    column groups 0/2, so a single [128, 512] DVE add folds val_cur in.
    NOTE: matmuls must write PSUM at column offset 0 — column-offset
    PSUM writes crash the hardware.
"""

import numpy as np

import concourse.bacc as bacc
import concourse.mybir as mybir
import concourse.tile as tile
from concourse import bass_utils, masks

F32 = mybir.dt.float32
BF16 = mybir.dt.bfloat16

N_CORES = 8
N, T, NTOK, C, C2 = 4, 8, 16384, 64, 64
NT1 = 512          # phase-1 128-token matmul tiles per core
NSL = 8192         # phase-2 token slice per core
HSL = NSL // 2     # 4096 tokens per phase-2 half
CHUNK_TILES = 32   # phase-1 tiles per DMA chunk (32 * 128 cols * 4B = 2 MiB)
N_CHUNKS = NT1 // CHUNK_TILES
PAIRS = [[0, 1], [2, 3], [4, 5], [6, 7]]

_CACHE = {}

# Extra kwargs forwarded to run_bass_kernel_spmd (used by the profiling
# harness to request an NTFF trace; empty for normal correctness runs).
_RUN_OPTS = {}


def _build_program():
    nc = bacc.Bacc(
        "TRN2",
        target_bir_lowering=False,
        debug=False,
        enable_asserts=False,
        num_devices=N_CORES,
    )

    kvp = nc.dram_tensor("kv_pack", [128, NT1 * 128], F32, kind="ExternalInput").ap()
    kct = nc.dram_tensor("key_curT", [2, C, HSL], F32, kind="ExternalInput").ap()
    vc = nc.dram_tensor("val_cur", [128, HSL], F32, kind="ExternalInput").ap()
    out = nc.dram_tensor("out", [128, HSL], F32, kind="ExternalOutput").ap()

    with tile.TileContext(nc) as tc:
        with (
            tc.tile_pool(name="persist", bufs=1) as persist,
            tc.tile_pool(name="big", bufs=5) as big,
            tc.tile_pool(name="tmp", bufs=2) as tmp,
            tc.tile_pool(name="ps", bufs=2, space="PSUM") as ps,
            tc.tile_pool(name="dram", bufs=1, space="DRAM") as dram,
        ):
            ident = persist.tile([128, 128], F32)
            masks.make_identity(nc, ident[:])

            kct_a = persist.tile([C, HSL], BF16)
            kct_b = persist.tile([C, HSL], BF16)
            vc_sb = persist.tile([128, HSL], F32)
            stage = persist.tile([128, HSL], F32)

            kvt_sb = persist.tile([C2, C], F32)
            kvt_all = persist.tile([C2, 2 * C], F32)
            kv_soft = persist.tile([C, C2], BF16)

            # ---- dummy collective: warm the ncfw/SPAD path early so the
            # real exchange doesn't pay first-use latency ----
            warm_in = dram.tile([C2, 1], F32, tag="warm_in", name="warm_in")
            warm_out = dram.tile([2, C2, 1], F32, tag="warm_out", name="warm_out")
            nc.gpsimd.dma_start(warm_in[:], ident[0:C2, 0:1])
            nc.gpsimd.collective_compute(
                "AllGather",
                mybir.AluOpType.bypass,
                replica_groups=PAIRS,
                ins=[warm_in.opt()],
                outs=[warm_out.opt()],
            )

            # ---- phase 1: partial kvT[d, c], col-tiled 2x ----
            kv_ps = ps.tile([128, C], F32, tag="kv", bufs=1)
            for ci in range(N_CHUNKS):
                q = nc.sync if ci % 2 == 0 else nc.scalar
                buf = big.tile([128, CHUNK_TILES * 128], F32, tag="k")
                lo = ci * CHUNK_TILES * 128
                q.dma_start(buf[:], kvp[:, lo:lo + CHUNK_TILES * 128])
                for la in range(CHUNK_TILES):
                    a = ci * CHUNK_TILES + la
                    half = a % 2
                    col = la * 128
                    nc.tensor.matmul(
                        kv_ps[64 * half:64 * half + C2, :],
                        lhsT=buf[:, col + 64:col + 128],
                        rhs=buf[:, col:col + 64],
                        start=(a < 2),
                        stop=(a >= NT1 - 2),
                        tile_position=(0, 64 * half),
                    )
            # phase-2 inputs: issued after the whole phase-1 chunk stream
            # so they never delay it; they ride the gpsimd (SWDGE) queue
            # and stream in during the collective wait. key_cur^T is cast
            # f32 -> bf16 during the DMA.
            nc.gpsimd.dma_start(kct_a[:], kct[0])
            nc.gpsimd.dma_start(kct_b[:], kct[1])
            nc.gpsimd.dma_start(vc_sb[:], vc)

            # partial kvT = even-half + odd-half (DVE reads only one PSUM
            # operand per instruction, so copy then add)
            nc.vector.tensor_copy(kvt_sb[:], kv_ps[0:C2, :])
            nc.vector.tensor_add(kvt_sb[:], kvt_sb[:], kv_ps[64:64 + C2, :])

            # ---- pair exchange: one 16 KB AllGather within each pair ----
            ar_in = dram.tile([C2, C], F32, tag="ar_in", name="ar_in")
            # pair groups (<=4 cores) require a Local (non-shared) output
            ar_out = dram.tile([2, C2, C], F32, tag="ar_out", name="ar_out")
            nc.sync.dma_start(ar_in[:], kvt_sb[:])
            nc.gpsimd.collective_compute(
                "AllGather",
                mybir.AluOpType.bypass,
                replica_groups=PAIRS,
                ins=[ar_in.opt()],
                outs=[ar_out.opt()],
            )
            nc.sync.dma_start(
                kvt_all[:].rearrange("d (r c) -> d r c", r=2),
                ar_out.rearrange("r d c -> d r c"),
            )
            kvt_red = tmp.tile([C2, C], F32)
            nc.vector.tensor_add(
                kvt_red[:], kvt_all[:, 0:C], kvt_all[:, C:2 * C]
            )

            # ---- softmax over c (free axis of kvT) ----
            neg_mx = tmp.tile([C2, 1], F32)
            nc.vector.reduce_max(
                out=neg_mx[:],
                in_=kvt_red[:],
                axis=mybir.AxisListType.X,
                negate=True,
            )
            ex = tmp.tile([C2, C], F32)
            sm = tmp.tile([C2, 1], F32)
            nc.scalar.activation(
                ex[:], kvt_red[:],
                mybir.ActivationFunctionType.Exp,
                bias=neg_mx[:], scale=1.0,
                accum_out=sm[:],
            )
            rv = tmp.tile([C2, 1], F32)
            nc.vector.reciprocal(rv[:], sm[:])
            nc.vector.tensor_scalar_mul(ex[:], ex[:], rv[:])
            # Transpose softmaxed kvT to kv[c, d] (transpose-mode matmul
            # must write PSUM partition 0); the DVE copy casts to bf16.
            tp = ps.tile([C, C2], F32, tag="tp", bufs=1)
            nc.tensor.transpose(tp[:], ex[:], ident[0:C2, 0:C2])
            nc.vector.tensor_copy(kv_soft[:], tp[:])

            # ---- phase 2: out^T[d, tok] = kv_soft^T @ key_cur^T + vc^T ----
            for s in range(8):
                pg = ps.tile([128, 512], F32, tag="o", name=f"o{s}", bufs=4)
                sl = slice(s * 512, (s + 1) * 512)
                nc.tensor.matmul(
                    pg[0:64, :],
                    lhsT=kv_soft[:],
                    rhs=kct_a[:, sl],
                    start=True, stop=True,
                    tile_position=(0, 0),
                )
                nc.tensor.matmul(
                    pg[64:128, :],
                    lhsT=kv_soft[:],
                    rhs=kct_b[:, sl],
                    start=True, stop=True,
                    tile_position=(0, 64),
                )
                nc.vector.tensor_add(stage[:, sl], pg[:], vc_sb[:, sl])
                # store each quarter as it completes; alternate queues so
                # stores overlap the remaining adds
                if s % 2 == 1:
                    q = nc.sync if s % 4 == 1 else nc.scalar
                    lo = (s - 1) * 512
                    q.dma_start(out[:, lo:lo + 1024], stage[:, lo:lo + 1024])

    nc.compile()
    return nc


def _get_program():
    if "nc" not in _CACHE:
        _CACHE["nc"] = _build_program()
    return _CACHE["nc"]


def kernel(key_mem, val_mem, key_cur, val_cur, alpha):
    key_mem = np.asarray(key_mem, dtype=np.float32)
    val_mem = np.asarray(val_mem, dtype=np.float32)
    key_cur = np.asarray(key_cur, dtype=np.float32)
    val_cur = np.asarray(val_cur, dtype=np.float32)
    alpha_f = float(np.asarray(alpha).reshape(-1)[0])

    nc = _get_program()

    kc_scaled = (alpha_f * key_cur).astype(np.float32)
    in_maps = []
    for i in range(N_CORES):
        b, h = i // 2, i % 2
        # phase-1 stream: interleave 128-token key/val tiles
        km = key_mem[b, 4 * h:4 * h + 4].reshape(NT1, 128, C)
        vm = val_mem[b, 4 * h:4 * h + 4].reshape(NT1, 128, C2)
        kv_pack = (
            np.concatenate([km, vm], axis=2)
            .transpose(1, 0, 2)
            .reshape(128, NT1 * 128)
        )
        # phase-2: key_cur^T (alpha folded) split into token halves A/B
        kc = kc_scaled[b, h * NSL:(h + 1) * NSL, :].T  # (C, NSL)
        kct_pack = np.stack([kc[:, 0:HSL], kc[:, HSL:NSL]])
        vcT = val_cur[b, h * NSL:(h + 1) * NSL, :].T  # (C2, NSL)
        vc_pack = np.concatenate([vcT[:, 0:HSL], vcT[:, HSL:NSL]], axis=0)
        in_maps.append(
            {
                "kv_pack": np.ascontiguousarray(kv_pack),
                "key_curT": np.ascontiguousarray(kct_pack),
                "val_cur": np.ascontiguousarray(vc_pack),
            }
        )

    res = bass_utils.run_bass_kernel_spmd(
        nc, in_maps, core_ids=list(range(N_CORES)), **_RUN_OPTS
    )
    _CACHE["last_result"] = res
    full = np.empty((N, NTOK, C2), dtype=np.float32)
    for i in range(N_CORES):
        b, h = i // 2, i % 2
        o = res.results[i]["out"]  # [128, HSL] = out^T halves stacked
        full[b, h * NSL:h * NSL + HSL, :] = o[0:C2].T
        full[b, h * NSL + HSL:(h + 1) * NSL, :] = o[C2:2 * C2].T
    return full


# revision 10
# speedup vs baseline: 1.6964x; 1.5037x over previous
"""ChannelAttentionPropagation1D kernel for 8x TRN2 NeuronCores.

Reference computation (per batch b):
  kv[c,d]   = sum_{t,n} key_mem[b,t,n,c] * val_mem[b,t,n,d]    # (64, 64)
  kv_soft   = softmax(kv, axis=c)
  out[n,d]  = alpha * (key_cur[b] @ kv_soft)[n,d] + val_cur[b,n,d]

Sharding (8 cores, pair-per-batch):
  core i owns batch b = i//2, token half h = i%2.
  phase 1: core i contracts its 65536 memory tokens into a partial
           kvT[d,c]; ONE pair AllGather (16 KB) merges the two halves.
  phase 2: core i computes the n-slice [h*8192, (h+1)*8192) of batch b.

Precision: phase-1 operands and key_cur^T are host-cast to fp16 —
  halves the HBM traffic (the kernel is memory-bound) and makes the PE
  single-pass instead of fp32's LOW/HIGH double pass. The kv logits
  have top1-top2 gaps of ~400 (median), so the softmax is insensitive
  to the ~0.5 absolute logit error fp16 introduces; measured end-to-end
  rel fro error ~1e-4 against an f64 reference (tolerance 2e-2).
  val_cur and all accumulations stay fp32.

Layout notes:
  - key/val memory tokens are host-interleaved into one packed fp16
    stream [128, 512*128] (per 128-token tile: 64 key cols then 64 val
    cols) so one DMA feeds both matmul operands; 2 MiB chunks alternate
    between the two HWDGE queues (sync / scalar).
  - phase 1 accumulates kvT[d,c] in PSUM col-tiled 2x (even tiles on PE
    column group 0, odd on group 2) so LDWEIGHTS/MATMUL overlap.
  - phase-2 inputs are queued on the HWDGE rings AFTER the last phase-1
    chunk (ring FIFO order guarantees they never delay the chunk
    stream); they stream in during the collective wait.
  - a dummy 256 B pair AllGather fires at kernel start to absorb the
    collective control-plane warmup (ncfw wakeup + SPAD staging); the
    real exchange then starts in ~1 us instead of ~11 us.
  - phase 2 computes out^T[d, tok] with kv_soft stationary (loaded once
    per column group) and key_cur^T as the N=512 moving operand; token
    halves A/B land on PSUM partitions 0:64 / 64:128 of one bank via
    column groups 0/2, so a single [128, 512] DVE add folds val_cur in.
    NOTE: matmuls must write PSUM at column offset 0 — column-offset
    PSUM writes crash the hardware.
"""

import numpy as np

import concourse.bacc as bacc
import concourse.mybir as mybir
import concourse.tile as tile
from concourse import bass_utils, masks

F32 = mybir.dt.float32
F16 = mybir.dt.float16

N_CORES = 8
N, T, NTOK, C, C2 = 4, 8, 16384, 64, 64
NT1 = 512          # phase-1 128-token matmul tiles per core
NSL = 8192         # phase-2 token slice per core
HSL = NSL // 2     # 4096 tokens per phase-2 half
CHUNK_TILES = 64   # phase-1 tiles per DMA chunk (64 * 128 cols * 2B = 2 MiB)
N_CHUNKS = NT1 // CHUNK_TILES
PAIRS = [[0, 1], [2, 3], [4, 5], [6, 7]]

_CACHE = {}

# Extra kwargs forwarded to run_bass_kernel_spmd (used by the profiling
# harness to request an NTFF trace; empty for normal correctness runs).
_RUN_OPTS = {}


def _build_program():
    nc = bacc.Bacc(
        "TRN2",
        target_bir_lowering=False,
        debug=False,
        enable_asserts=False,
        num_devices=N_CORES,
    )

    kvp = nc.dram_tensor("kv_pack", [128, NT1 * 128], F16, kind="ExternalInput").ap()
    kct = nc.dram_tensor("key_curT", [2, C, HSL], F16, kind="ExternalInput").ap()
    vc = nc.dram_tensor("val_cur", [128, HSL], F32, kind="ExternalInput").ap()
    out = nc.dram_tensor("out", [128, HSL], F32, kind="ExternalOutput").ap()

    with tile.TileContext(nc) as tc:
        with (
            tc.tile_pool(name="persist", bufs=1) as persist,
            tc.tile_pool(name="big", bufs=4) as big,
            tc.tile_pool(name="tmp", bufs=2) as tmp,
            tc.tile_pool(name="ps", bufs=2, space="PSUM") as ps,
            tc.tile_pool(name="dram", bufs=1, space="DRAM") as dram,
        ):
            ident = persist.tile([128, 128], F32)
            masks.make_identity(nc, ident[:])

            kct_a = persist.tile([C, HSL], F16)
            kct_b = persist.tile([C, HSL], F16)
            vc_sb = persist.tile([128, HSL], F32)
            stage = persist.tile([128, HSL], F32)

            kvt_sb = persist.tile([C2, C], F32)
            kvt_all = persist.tile([C2, 2 * C], F32)
            kv_soft = persist.tile([C, C2], F16)

            # ---- dummy collective: warm the ncfw/SPAD path early so the
            # real exchange doesn't pay first-use latency ----
            warm_in = dram.tile([C2, 1], F32, tag="warm_in", name="warm_in")
            warm_out = dram.tile([2, C2, 1], F32, tag="warm_out", name="warm_out")
            nc.gpsimd.dma_start(warm_in[:], ident[0:C2, 0:1])
            nc.gpsimd.collective_compute(
                "AllGather",
                mybir.AluOpType.bypass,
                replica_groups=PAIRS,
                ins=[warm_in.opt()],
                outs=[warm_out.opt()],
            )

            # ---- phase 1: partial kvT[d, c], col-tiled 2x ----
            kv_ps = ps.tile([128, C], F32, tag="kv", bufs=1)
            for ci in range(N_CHUNKS):
                q = nc.sync if ci % 2 == 0 else nc.scalar
                buf = big.tile([128, CHUNK_TILES * 128], F16, tag="k")
                lo = ci * CHUNK_TILES * 128
                q.dma_start(buf[:], kvp[:, lo:lo + CHUNK_TILES * 128])
                for la in range(CHUNK_TILES):
                    a = ci * CHUNK_TILES + la
                    half = a % 2
                    col = la * 128
                    nc.tensor.matmul(
                        kv_ps[64 * half:64 * half + C2, :],
                        lhsT=buf[:, col + 64:col + 128],
                        rhs=buf[:, col:col + 64],
                        start=(a < 2),
                        stop=(a >= NT1 - 2),
                        tile_position=(0, 64 * half),
                    )
            # phase-2 inputs: queued on the HWDGE rings behind the last
            # chunks (FIFO order), so they stream during the collective
            # wait without delaying phase 1.
            nc.sync.dma_start(kct_a[:], kct[0])
            nc.sync.dma_start(kct_b[:], kct[1])
            nc.scalar.dma_start(vc_sb[:], vc)

            # partial kvT = even-half + odd-half (DVE reads only one PSUM
            # operand per instruction, so copy then add)
            nc.vector.tensor_copy(kvt_sb[:], kv_ps[0:C2, :])
            nc.vector.tensor_add(kvt_sb[:], kvt_sb[:], kv_ps[64:64 + C2, :])

            # ---- pair exchange: one 16 KB AllGather within each pair ----
            ar_in = dram.tile([C2, C], F32, tag="ar_in", name="ar_in")
            # pair groups (<=4 cores) require a Local (non-shared) output
            ar_out = dram.tile([2, C2, C], F32, tag="ar_out", name="ar_out")
            nc.sync.dma_start(ar_in[:], kvt_sb[:])
            nc.gpsimd.collective_compute(
                "AllGather",
                mybir.AluOpType.bypass,
                replica_groups=PAIRS,
                ins=[ar_in.opt()],
                outs=[ar_out.opt()],
            )
            nc.sync.dma_start(
                kvt_all[:].rearrange("d (r c) -> d r c", r=2),
                ar_out.rearrange("r d c -> d r c"),
            )
            kvt_red = tmp.tile([C2, C], F32)
            nc.vector.tensor_add(
                kvt_red[:], kvt_all[:, 0:C], kvt_all[:, C:2 * C]
            )

            # ---- softmax over c (free axis of kvT) ----
            neg_mx = tmp.tile([C2, 1], F32)
            nc.vector.reduce_max(
                out=neg_mx[:],
                in_=kvt_red[:],
                axis=mybir.AxisListType.X,
                negate=True,
            )
            ex = tmp.tile([C2, C], F32)
            sm = tmp.tile([C2, 1], F32)
            nc.scalar.activation(
                ex[:], kvt_red[:],
                mybir.ActivationFunctionType.Exp,
                bias=neg_mx[:], scale=1.0,
                accum_out=sm[:],
            )
            rv = tmp.tile([C2, 1], F32)
            nc.vector.reciprocal(rv[:], sm[:])
            nc.vector.tensor_scalar_mul(ex[:], ex[:], rv[:])
            # Transpose softmaxed kvT to kv[c, d] (transpose-mode matmul
            # must write PSUM partition 0); the DVE copy casts to fp16.
            tp = ps.tile([C, C2], F32, tag="tp", bufs=1)
            nc.tensor.transpose(tp[:], ex[:], ident[0:C2, 0:C2])
            nc.vector.tensor_copy(kv_soft[:], tp[:])

            # ---- phase 2: out^T[d, tok] = kv_soft^T @ key_cur^T + vc^T ----
            for s in range(8):
                pg = ps.tile([128, 512], F32, tag="o", name=f"o{s}", bufs=4)
                sl = slice(s * 512, (s + 1) * 512)
                nc.tensor.matmul(
                    pg[0:64, :],
                    lhsT=kv_soft[:],
                    rhs=kct_a[:, sl],
                    start=True, stop=True,
                    tile_position=(0, 0),
                )
                nc.tensor.matmul(
                    pg[64:128, :],
                    lhsT=kv_soft[:],
                    rhs=kct_b[:, sl],
                    start=True, stop=True,
                    tile_position=(0, 64),
                )
                nc.vector.tensor_add(stage[:, sl], pg[:], vc_sb[:, sl])
                # store each quarter as it completes; alternate queues so
                # stores overlap the remaining adds
                if s % 2 == 1:
                    q = nc.sync if s % 4 == 1 else nc.scalar
                    lo = (s - 1) * 512
                    q.dma_start(out[:, lo:lo + 1024], stage[:, lo:lo + 1024])

    nc.compile()
    return nc


def _get_program():
    if "nc" not in _CACHE:
        _CACHE["nc"] = _build_program()
    return _CACHE["nc"]


def kernel(key_mem, val_mem, key_cur, val_cur, alpha):
    key_mem = np.asarray(key_mem, dtype=np.float32)
    val_mem = np.asarray(val_mem, dtype=np.float32)
    key_cur = np.asarray(key_cur, dtype=np.float32)
    val_cur = np.asarray(val_cur, dtype=np.float32)
    alpha_f = float(np.asarray(alpha).reshape(-1)[0])

    nc = _get_program()

    kc_scaled = (alpha_f * key_cur).astype(np.float32)
    in_maps = []
    for i in range(N_CORES):
        b, h = i // 2, i % 2
        # phase-1 stream: interleave 128-token key/val tiles (fp16)
        km = key_mem[b, 4 * h:4 * h + 4].reshape(NT1, 128, C)
        vm = val_mem[b, 4 * h:4 * h + 4].reshape(NT1, 128, C2)
        kv_pack = (
            np.concatenate([km, vm], axis=2)
            .transpose(1, 0, 2)
            .reshape(128, NT1 * 128)
            .astype(np.float16)
        )
        # phase-2: key_cur^T (alpha folded, fp16) split into halves A/B
        kc = kc_scaled[b, h * NSL:(h + 1) * NSL, :].T  # (C, NSL)
        kct_pack = np.stack([kc[:, 0:HSL], kc[:, HSL:NSL]]).astype(np.float16)
        vcT = val_cur[b, h * NSL:(h + 1) * NSL, :].T  # (C2, NSL)
        vc_pack = np.concatenate([vcT[:, 0:HSL], vcT[:, HSL:NSL]], axis=0)
        in_maps.append(
            {
                "kv_pack": np.ascontiguousarray(kv_pack),
                "key_curT": np.ascontiguousarray(kct_pack),
                "val_cur": np.ascontiguousarray(vc_pack),
            }
        )

    res = bass_utils.run_bass_kernel_spmd(
        nc, in_maps, core_ids=list(range(N_CORES)), **_RUN_OPTS
    )
    _CACHE["last_result"] = res
    full = np.empty((N, NTOK, C2), dtype=np.float32)
    for i in range(N_CORES):
        b, h = i // 2, i % 2
        o = res.results[i]["out"]  # [128, HSL] = out^T halves stacked
        full[b, h * NSL:h * NSL + HSL, :] = o[0:C2].T
        full[b, h * NSL + HSL:(h + 1) * NSL, :] = o[C2:2 * C2].T
    return full


# revision 12
# speedup vs baseline: 1.7212x; 1.0146x over previous
"""ChannelAttentionPropagation1D kernel for 8x TRN2 NeuronCores.

Reference computation (per batch b):
  kv[c,d]   = sum_{t,n} key_mem[b,t,n,c] * val_mem[b,t,n,d]    # (64, 64)
  kv_soft   = softmax(kv, axis=c)
  out[n,d]  = alpha * (key_cur[b] @ kv_soft)[n,d] + val_cur[b,n,d]

Sharding (8 cores, pair-per-batch):
  core i owns batch b = i//2, token half h = i%2.
  phase 1: core i contracts its 65536 memory tokens into a partial
           kvT[d,c]; ONE pair AllGather (16 KB) merges the two halves.
  phase 2: core i computes the n-slice [h*8192, (h+1)*8192) of batch b.

Precision: phase-1 operands and key_cur^T are host-cast to fp16 —
  halves the HBM traffic (the kernel is memory-bound) and makes the PE
  single-pass instead of fp32's LOW/HIGH double pass. The kv logits
  have top1-top2 gaps of ~400 (median), so the softmax is insensitive
  to the ~0.5 absolute logit error fp16 introduces; measured end-to-end
  rel fro error ~1e-4 against an f64 reference (tolerance 2e-2).
  val_cur and all accumulations stay fp32.

Layout notes:
  - key/val memory tokens are host-interleaved into one packed fp16
    stream [128, 512*128] (per 128-token tile: 64 key cols then 64 val
    cols) so one DMA feeds both matmul operands; 2 MiB chunks alternate
    between the two HWDGE queues (sync / scalar).
  - phase 1 accumulates kvT[d,c] in PSUM col-tiled 2x (even tiles on PE
    column group 0, odd on group 2) so LDWEIGHTS/MATMUL overlap.
  - phase-2 inputs are queued on the HWDGE rings AFTER the last phase-1
    chunk (ring FIFO order guarantees they never delay the chunk
    stream); they stream in during the collective wait.
  - a dummy 256 B pair AllGather fires at kernel start to absorb the
    collective control-plane warmup (ncfw wakeup + SPAD staging); the
    real exchange then starts in ~1 us instead of ~11 us.
  - phase 2 computes out^T[d, tok] with kv_soft stationary (loaded once
    per column group) and key_cur^T as the N=512 moving operand; token
    halves A/B land on PSUM partitions 0:64 / 64:128 of one bank via
    column groups 0/2, so a single [128, 512] DVE add folds val_cur in.
    NOTE: matmuls must write PSUM at column offset 0 — column-offset
    PSUM writes crash the hardware.
"""

import numpy as np

import concourse.bacc as bacc
import concourse.mybir as mybir
import concourse.tile as tile
from concourse import bass_utils, masks

F32 = mybir.dt.float32
F16 = mybir.dt.float16

N_CORES = 8
N, T, NTOK, C, C2 = 4, 8, 16384, 64, 64
NT1 = 512          # phase-1 128-token matmul tiles per core
NSL = 8192         # phase-2 token slice per core
HSL = NSL // 2     # 4096 tokens per phase-2 half
CHUNK_TILES = 64   # phase-1 tiles per DMA chunk (64 * 128 cols * 2B = 2 MiB)
N_CHUNKS = NT1 // CHUNK_TILES
PAIRS = [[0, 1], [2, 3], [4, 5], [6, 7]]

_CACHE = {}

# Extra kwargs forwarded to run_bass_kernel_spmd (used by the profiling
# harness to request an NTFF trace; empty for normal correctness runs).
_RUN_OPTS = {}


def _build_program():
    nc = bacc.Bacc(
        "TRN2",
        target_bir_lowering=False,
        debug=False,
        enable_asserts=False,
        num_devices=N_CORES,
    )

    kvp = nc.dram_tensor("kv_pack", [128, NT1 * 128], F16, kind="ExternalInput").ap()
    kct = nc.dram_tensor("key_curT", [2, C, HSL], F16, kind="ExternalInput").ap()
    vc = nc.dram_tensor("val_cur", [128, HSL], F32, kind="ExternalInput").ap()
    out = nc.dram_tensor("out", [128, HSL], F32, kind="ExternalOutput").ap()

    with tile.TileContext(nc) as tc:
        with (
            tc.tile_pool(name="persist", bufs=1) as persist,
            tc.tile_pool(name="big", bufs=4) as big,
            tc.tile_pool(name="tmp", bufs=2) as tmp,
            tc.tile_pool(name="ps", bufs=2, space="PSUM") as ps,
            tc.tile_pool(name="dram", bufs=1, space="DRAM") as dram,
        ):
            ident = persist.tile([128, 128], F32)
            masks.make_identity(nc, ident[:])

            kct_a = persist.tile([C, HSL], F16)
            kct_b = persist.tile([C, HSL], F16)
            vc_sb = persist.tile([128, HSL], F32)
            stage = persist.tile([128, HSL], F32)

            kvt_sb = persist.tile([C2, C], F32)
            kvt_all = persist.tile([C2, 2 * C], F32)
            kv_soft = persist.tile([C, C2], F16)

            # ---- dummy collective: warm the ncfw/SPAD path early so the
            # real exchange doesn't pay first-use latency ----
            warm_in = dram.tile([C2, 1], F32, tag="warm_in", name="warm_in")
            warm_out = dram.tile([2, C2, 1], F32, tag="warm_out", name="warm_out")
            nc.gpsimd.dma_start(warm_in[:], ident[0:C2, 0:1])
            nc.gpsimd.collective_compute(
                "AllGather",
                mybir.AluOpType.bypass,
                replica_groups=PAIRS,
                ins=[warm_in.opt()],
                outs=[warm_out.opt()],
            )

            # ---- phase 1: partial kvT[d, c], col-tiled 2x ----
            kv_ps = ps.tile([128, C], F32, tag="kv", bufs=1)
            last_buf = {}
            for ci in range(N_CHUNKS):
                q = nc.sync if ci % 2 == 0 else nc.scalar
                buf = big.tile([128, CHUNK_TILES * 128], F16, tag="k")
                last_buf[ci % 2] = buf
                lo = ci * CHUNK_TILES * 128
                q.dma_start(buf[:], kvp[:, lo:lo + CHUNK_TILES * 128])
                for la in range(CHUNK_TILES):
                    a = ci * CHUNK_TILES + la
                    half = a % 2
                    col = la * 128
                    nc.tensor.matmul(
                        kv_ps[64 * half:64 * half + C2, :],
                        lhsT=buf[:, col + 64:col + 128],
                        rhs=buf[:, col:col + 64],
                        start=(a < 2),
                        stop=(a >= NT1 - 2),
                        tile_position=(0, 64 * half),
                    )
            # partial kvT = even-half + odd-half (DVE reads only one PSUM
            # operand per instruction, so copy then add)
            nc.vector.tensor_copy(kvt_sb[:], kv_ps[0:C2, :])
            nc.vector.tensor_add(kvt_sb[:], kvt_sb[:], kv_ps[64:64 + C2, :])

            # phase-2 inputs on the HWDGE rings, pinned BEHIND the last
            # phase-1 chunks with tiny copies (Tile otherwise hoists
            # dependency-free DMAs ahead of the chunk stream); they then
            # stream during the collective wait.
            nc.vector.tensor_copy(kct_a[0:1, 0:1], last_buf[0][0:1, 0:1])
            nc.vector.tensor_copy(kct_b[0:1, 0:1], last_buf[0][0:1, 0:1])
            nc.vector.tensor_copy(vc_sb[0:1, 0:1], last_buf[1][0:1, 0:1])
            ar_in = dram.tile([C2, C], F32, tag="ar_in", name="ar_in")
            nc.sync.dma_start(ar_in[:], kvt_sb[:])
            nc.sync.dma_start(kct_a[:], kct[0])
            nc.sync.dma_start(kct_b[:], kct[1])
            nc.scalar.dma_start(vc_sb[:], vc)

            # ---- pair exchange: one 16 KB AllGather within each pair ----
            # pair groups (<=4 cores) require a Local (non-shared) output
            ar_out = dram.tile([2, C2, C], F32, tag="ar_out", name="ar_out")
            nc.gpsimd.collective_compute(
                "AllGather",
                mybir.AluOpType.bypass,
                replica_groups=PAIRS,
                ins=[ar_in.opt()],
                outs=[ar_out.opt()],
            )
            # readback rides the gpsimd (SWDGE) queue: the Q7 is blocked on
            # the collective trigger anyway, so the readback issues the
            # moment the collective completes — and its semaphore wait
            # cannot stall the HWDGE rings carrying kct/vc/stores.
            nc.gpsimd.dma_start(
                kvt_all[:].rearrange("d (r c) -> d r c", r=2),
                ar_out.rearrange("r d c -> d r c"),
            )
            kvt_red = tmp.tile([C2, C], F32)
            nc.vector.tensor_add(
                kvt_red[:], kvt_all[:, 0:C], kvt_all[:, C:2 * C]
            )

            # ---- softmax over c (free axis of kvT) ----
            neg_mx = tmp.tile([C2, 1], F32)
            nc.vector.reduce_max(
                out=neg_mx[:],
                in_=kvt_red[:],
                axis=mybir.AxisListType.X,
                negate=True,
            )
            ex = tmp.tile([C2, C], F32)
            sm = tmp.tile([C2, 1], F32)
            nc.scalar.activation(
                ex[:], kvt_red[:],
                mybir.ActivationFunctionType.Exp,
                bias=neg_mx[:], scale=1.0,
                accum_out=sm[:],
            )
            rv = tmp.tile([C2, 1], F32)
            nc.vector.reciprocal(rv[:], sm[:])
            nc.vector.tensor_scalar_mul(ex[:], ex[:], rv[:])
            # Transpose softmaxed kvT to kv[c, d] (transpose-mode matmul
            # must write PSUM partition 0); the DVE copy casts to fp16.
            tp = ps.tile([C, C2], F32, tag="tp", bufs=1)
            nc.tensor.transpose(tp[:], ex[:], ident[0:C2, 0:C2])
            nc.vector.tensor_copy(kv_soft[:], tp[:])

            # ---- phase 2: out^T[d, tok] = kv_soft^T @ key_cur^T + vc^T ----
            for s in range(8):
                pg = ps.tile([128, 512], F32, tag="o", name=f"o{s}", bufs=4)
                sl = slice(s * 512, (s + 1) * 512)
                nc.tensor.matmul(
                    pg[0:64, :],
                    lhsT=kv_soft[:],
                    rhs=kct_a[:, sl],
                    start=True, stop=True,
                    tile_position=(0, 0),
                )
                nc.tensor.matmul(
                    pg[64:128, :],
                    lhsT=kv_soft[:],
                    rhs=kct_b[:, sl],
                    start=True, stop=True,
                    tile_position=(0, 64),
                )
                nc.vector.tensor_add(stage[:, sl], pg[:], vc_sb[:, sl])
                # store each quarter as it completes; alternate queues so
                # stores overlap the remaining adds
                if s % 2 == 1:
                    q = nc.sync if s % 4 == 1 else nc.scalar
                    lo = (s - 1) * 512
                    q.dma_start(out[:, lo:lo + 1024], stage[:, lo:lo + 1024])

    nc.compile()
    return nc


def _get_program():
    if "nc" not in _CACHE:
        _CACHE["nc"] = _build_program()
    return _CACHE["nc"]


def kernel(key_mem, val_mem, key_cur, val_cur, alpha):
    key_mem = np.asarray(key_mem, dtype=np.float32)
    val_mem = np.asarray(val_mem, dtype=np.float32)
    key_cur = np.asarray(key_cur, dtype=np.float32)
    val_cur = np.asarray(val_cur, dtype=np.float32)
    alpha_f = float(np.asarray(alpha).reshape(-1)[0])

    nc = _get_program()

    kc_scaled = (alpha_f * key_cur).astype(np.float32)
    in_maps = []
    for i in range(N_CORES):
        b, h = i // 2, i % 2
        # phase-1 stream: interleave 128-token key/val tiles (fp16)
        km = key_mem[b, 4 * h:4 * h + 4].reshape(NT1, 128, C)
        vm = val_mem[b, 4 * h:4 * h + 4].reshape(NT1, 128, C2)
        kv_pack = (
            np.concatenate([km, vm], axis=2)
            .transpose(1, 0, 2)
            .reshape(128, NT1 * 128)
            .astype(np.float16)
        )
        # phase-2: key_cur^T (alpha folded, fp16) split into halves A/B
        kc = kc_scaled[b, h * NSL:(h + 1) * NSL, :].T  # (C, NSL)
        kct_pack = np.stack([kc[:, 0:HSL], kc[:, HSL:NSL]]).astype(np.float16)
        vcT = val_cur[b, h * NSL:(h + 1) * NSL, :].T  # (C2, NSL)
        vc_pack = np.concatenate([vcT[:, 0:HSL], vcT[:, HSL:NSL]], axis=0)
        in_maps.append(
            {
                "kv_pack": np.ascontiguousarray(kv_pack),
                "key_curT": np.ascontiguousarray(kct_pack),
                "val_cur": np.ascontiguousarray(vc_pack),
            }
        )

    res = bass_utils.run_bass_kernel_spmd(
        nc, in_maps, core_ids=list(range(N_CORES)), **_RUN_OPTS
    )
    _CACHE["last_result"] = res
    full = np.empty((N, NTOK, C2), dtype=np.float32)
    for i in range(N_CORES):
        b, h = i // 2, i % 2
        o = res.results[i]["out"]  # [128, HSL] = out^T halves stacked
        full[b, h * NSL:h * NSL + HSL, :] = o[0:C2].T
        full[b, h * NSL + HSL:(h + 1) * NSL, :] = o[C2:2 * C2].T
    return full
